# revision 22
# baseline (speedup 1.0000x reference)
"""Trainium2 Bass kernel for LocalFeatureAggregation (gnn_message_passing).

Sharding: data-parallel over points. Each of the 8 cores gets 12500 points
(padded to 12544 = 98 tiles of 128). All neighbor geometry (rel, dist, orig,
nbr, ones-bias-row) is precomputed on host and shipped as an 11-row tensor
per tile; MLP weights replicated.

Math decomposition (same as validated v1):
  lse+pool(x, f): z = [enc | f], softmax over channels, sum over K.
    E=exp(z); s_k = sum_c E; Ef=exp(f); Sf=sum_c Ef; r_k = 1/(s_k+Sf)
    pool_enc = sum_k z*E*r_k ; pool_f = (f*Ef) * sum_k r_k

v2 structure:
  - geo [11,K,T] from host (incl dist + ones row) -> L1 of BOTH encoders
    fused into one [11->128] matmul pair + one relu per tile.
  - z as a single [128,16,128] f32 PSUM tile (16 matmuls + 4 bias matmuls).
  - one exp over the full z; Sf via activation accum_out.
  - per-k tensor_scalar r-scaling (4x DVE mode); k-sum tree split
    DVE/Pool; f-half via fused scalar_tensor_tensor.
  - final leaky via Prelu activation (bias=b2s, alpha=0.01).
  - all activation funcs in one table set (exp_and_others) -> 1 table load.
"""
import os
import numpy as np
import ml_dtypes

import concourse.bass as bass
import concourse.bacc as bacc
import concourse.mybir as mybir
import concourse.tile as tile
from concourse.bass_utils import run_bass_kernel_spmd
from concourse.masks import make_identity

AF = mybir.ActivationFunctionType
ALU = mybir.AluOpType
AX = mybir.AxisListType
F32 = mybir.dt.float32
BF16 = mybir.dt.bfloat16
BF = ml_dtypes.bfloat16

N_FULL = 100000
NCORES = 8
NSH = N_FULL // NCORES      # 12500
T = 128                     # points per tile
K = 16                      # neighbors
S = 7                       # tiles per supertile
D_IN = 128
D_OUT = 256

# engine knobs for flexible ops
KN_IDENT = "act"            # feature-path identity moves: 'act'|'dve'
KN_KSUM = "dma"             # k-sum: 'dma' (gpsimd accum dma) | 'pool' tree
CTBK = 4                    # ctb k-slices on DVE (rest on Pool)


def _ceil_to(x, m):
    return ((x + m - 1) // m) * m


def _bcast_last(ap2d, last):
    """[P, F] AP -> [P, F, last] AP with step-0 trailing dim."""
    a = ap2d.rearrange("p (f a) -> p f a", a=1)
    return a.to_broadcast([ap2d.shape[0], ap2d.shape[1], last])


def build(nsh_pad):
    assert nsh_pad % (T * S) == 0
    nt = nsh_pad // T           # tiles
    nst = nt // S               # supertiles
    FW = S * T                  # 896
    CH = [(0, 448), (448, 448)]

    nc = bacc.Bacc(trn_type="TRN2")

    featT = nc.dram_tensor("featT", [D_IN, nsh_pad], BF16, kind="ExternalInput")
    geo4 = nc.dram_tensor("geo4", [nt, 11, K, T], BF16, kind="ExternalInput")
    out_d = nc.dram_tensor("out", [128, 4, nst, FW], BF16,
                           kind="ExternalOutput")

    def din(name, shape, dt):
        return nc.dram_tensor(name, shape, dt, kind="ExternalInput")

    wd = {}
    wd["w1a_cat"] = din("w1a_cat", [11, 128], BF16)
    wd["w2a_cat"] = din("w2a_cat", [128, 128], BF16)
    for ln in ("l1", "l2"):
        wd[ln + "_b2q"] = din(ln + "_b2q", [1, 512], BF16)
    wd["m1_W1"] = din("m1_W1", [128, 128], BF16)
    wd["m1_W2"] = din("m1_W2", [128, 128], BF16)
    wd["m1_b1"] = din("m1_b1", [128, 1], F32)
    wd["m1_b2"] = din("m1_b2", [128, 1], F32)
    wd["p1_W1"] = din("p1_W1", [128, 512], BF16)
    wd["p1_b1"] = din("p1_b1", [128, 2], F32)
    wd["p1_W2"] = din("p1_W2", [128, 256], BF16)
    wd["p1_b2"] = din("p1_b2", [128, 1], F32)
    wd["p2_W1"] = din("p2_W1", [128, 512], BF16)
    wd["p2_b1"] = din("p2_b1", [128, 2], F32)
    wd["p2_W2"] = din("p2_W2", [128, 512], BF16)
    wd["p2_b2"] = din("p2_b2", [128, 2], F32)
    wd["m2_W1"] = din("m2_W1", [128, 768], BF16)
    wd["m2_b1"] = din("m2_b1", [128, 3], F32)
    wd["m2_W2"] = din("m2_W2", [128, 1536], BF16)
    wd["m3_W1"] = din("m3_W1", [128, 256], BF16)
    wd["m3_b1"] = din("m3_b1", [128, 2], F32)
    wd["m3_W2"] = din("m3_W2", [128, 1024], BF16)
    wd["b2s"] = din("b2s", [128, 4], F32)

    with tile.TileContext(nc) as tc:
        import contextlib
        with contextlib.ExitStack() as ctx:
            cst = ctx.enter_context(tc.tile_pool(name="cst", bufs=1))
            geo_p = ctx.enter_context(tc.tile_pool(name="geo", bufs=4))
            h_p = ctx.enter_context(tc.tile_pool(name="hp", bufs=8))
            enc_p = ctx.enter_context(tc.tile_pool(name="enc", bufs=2))
            pool_p = ctx.enter_context(tc.tile_pool(name="pool", bufs=3))
            feat_p = ctx.enter_context(tc.tile_pool(name="feat", bufs=2))
            ps_z = ctx.enter_context(
                tc.tile_pool(name="psz", bufs=1, space="PSUM"))
            ps_h = ctx.enter_context(
                tc.tile_pool(name="psh", bufs=1, space="PSUM"))
            ps_m = ctx.enter_context(
                tc.tile_pool(name="psm", bufs=2, space="PSUM"))

            # ---------- constants ----------
            w_sb = {}
            for name, dr in wd.items():
                t_ = cst.tile(list(dr.shape), dr.dtype, name="w_" + name)
                nc.sync.dma_start(out=t_[:], in_=dr[:])
                w_sb[name] = t_
            ones_col = cst.tile([1, 128], BF16, name="ones_col")
            nc.gpsimd.memset(ones_col[:], 1.0)

            def fmlp(rhs_tiles, wkey, bkey, nci, ncout, kind, out_t,
                     extra=None):
                """Channel-major MLP layer over the supertile.
                rhs_tiles: list of nci tiles [128, FW]; out_t [128, nblk, FW].
                kind: 'relu' | 'ident' | 'prelu'.
                extra: (rhs_tiles2, wkey2, nci2) accumulated into same psum."""
                w = w_sb[wkey]
                b = w_sb[bkey]
                nblk = ncout // 128
                for co in range(nblk):
                    for c0, cw in CH:
                        ps = ps_m.tile([128, 448], F32, tag="misc", name="ps_f")
                        tot = nci + (extra[2] if extra else 0)
                        i = 0
                        for ci in range(nci):
                            nc.tensor.matmul(
                                out=ps[:, :cw],
                                lhsT=w[:, ci * ncout + co * 128:
                                       ci * ncout + (co + 1) * 128],
                                rhs=rhs_tiles[ci][:, c0:c0 + cw],
                                start=(i == 0), stop=(i == tot - 1))
                            i += 1
                        if extra:
                            rhs2, wkey2, nci2 = extra
                            w2 = w_sb[wkey2]
                            for ci in range(nci2):
                                nc.tensor.matmul(
                                    out=ps[:, :cw],
                                    lhsT=w2[:, ci * ncout + co * 128:
                                            ci * ncout + (co + 1) * 128],
                                    rhs=rhs2[ci][:, c0:c0 + cw],
                                    start=(i == 0), stop=(i == tot - 1))
                                i += 1
                        dst = out_t[:, co, c0:c0 + cw]
                        if kind == "relu":
                            nc.scalar.activation(
                                out=dst, in_=ps[:, :cw], func=AF.Relu,
                                bias=b[:, co:co + 1])
                        elif kind == "prelu":
                            nc.scalar.activation(
                                out=dst, in_=ps[:, :cw], func=AF.Prelu,
                                bias=b[:, co:co + 1], alpha=0.01)
                        elif KN_IDENT == "act":
                            nc.scalar.activation(
                                out=dst, in_=ps[:, :cw], func=AF.Identity,
                                bias=b[:, co:co + 1])
                        else:
                            nc.vector.tensor_scalar_add(
                                out=dst, in0=ps[:, :cw],
                                scalar1=b[:, co:co + 1])

            def enc_l1(geo_t, h_t):
                """Fused L1 for both encoders: h_t [128, K, T] bf16.
                Rows 0-63 = l1 hidden, 64-127 = l2 hidden."""
                for half in range(2):
                    ph = ps_h.tile([128, 8, T], F32, tag="ph", name="ph")
                    for q in range(2):
                        nc.tensor.matmul(
                            out=ph[:, 4 * q:4 * q + 4, :],
                            lhsT=w_sb["w1a_cat"][:],
                            rhs=geo_t[:, 8 * half + 4 * q:
                                      8 * half + 4 * q + 4, :])
                    nc.scalar.activation(
                        out=h_t[:, 8 * half:8 * half + 8, :], in_=ph[:],
                        func=AF.Relu)

            def pool_out(res, t_in_s, pcm_lo, pcm_hi):
                """XBAR-transpose pool rows -> channel-major dest slices."""
                M, prf = res
                dst = pcm_lo[:, t_in_s * T:(t_in_s + 1) * T]
                nc.sync.dma_start(out=dst, in_=M[:, 0, :], transpose=True)
                dst = pcm_hi[:, t_in_s * T:(t_in_s + 1) * T]
                nc.sync.dma_start(out=dst, in_=prf[:], transpose=True)

            def enc_pool(lname, half, xcm, t_in_s, h_t):
                """Encoder L2 + attentive pool for one tile.
                Returns (M, prf): M[:,0,:] = enc-half pool row (point-major),
                prf = f-half pool row."""
                w2a = w_sb["w2a_cat"][64 * half:64 * half + 64, :]
                b2q = w_sb[lname + "_b2q"]
                hh = h_t[64 * half:64 * half + 64, :, :]
                # f-half first: XBAR transpose into SBUF, Ef overlaps L2
                fT = pool_p.tile([T, 128], BF16, tag="fT", name="fT")
                nc.sync.dma_start(
                    out=fT[:], in_=xcm[:, t_in_s * T:(t_in_s + 1) * T],
                    transpose=True)
                Ef = pool_p.tile([T, 128], BF16, tag="Ef", name="Ef")
                Sf = pool_p.tile([T, 1], F32, tag="Sf", name="Sf")
                with nc.allow_low_precision("accum f32 ok"):
                    nc.scalar.activation(out=Ef[:], in_=fT[:], func=AF.Exp,
                                         accum_out=Sf[:])
                E = enc_p.tile([T, K, 128], BF16, tag="E", name="E")
                M = enc_p.tile([T, K, 128], BF16, tag="M", name="M", bufs=4)
                for hf in range(2):
                    z = ps_z.tile([T, 8, 128], F32, tag=f"z{hf}", name="z")
                    for k in range(8):
                        nc.tensor.matmul(
                            out=z[:, k, :], lhsT=hh[:, 8 * hf + k, :],
                            rhs=w2a, start=True, stop=False,
                            skip_group_check=True)
                    for g in range(2):
                        nc.tensor.matmul(
                            out=z[:, 4 * g:4 * g + 4, :], lhsT=ones_col[:],
                            rhs=b2q[:], start=False, stop=True,
                            skip_group_check=True)
                    sl = slice(8 * hf, 8 * hf + 8)
                    nc.scalar.activation(out=E[:, sl, :], in_=z[:],
                                         func=AF.Exp)
                    nc.vector.tensor_mul(out=M[:, sl, :], in0=E[:, sl, :],
                                         in1=z[:])
                # s_k = sum_c E via two bf16 halvings + short reduce
                s = pool_p.tile([T, K], BF16, tag="s", name="s")
                sc = pool_p.tile([T, K, 64], BF16, tag="sc", name="sc")
                with nc.allow_low_precision("bf16 softmax denom"):
                    nc.vector.tensor_add(out=sc[:], in0=E[:, :, 0:64],
                                         in1=E[:, :, 64:128])
                    nc.vector.tensor_add(out=sc[:, :, 0:32],
                                         in0=sc[:, :, 0:32],
                                         in1=sc[:, :, 32:64])
                    nc.vector.reduce_sum(out=s[:], in_=sc[:, :, 0:32],
                                         axis=AX.X)
                u = pool_p.tile([T, K], BF16, tag="u", name="u")
                with nc.allow_low_precision("bf16 softmax denom"):
                    nc.scalar.activation(out=u[:], in_=s[:],
                                         func=AF.Identity, bias=Sf[:])
                r = pool_p.tile([T, K], F32, tag="r", name="r")
                nc.vector.reciprocal(out=r[:], in_=u[:])
                sr = pool_p.tile([T, 1], F32, tag="sr", name="sr")
                nc.vector.reduce_sum(out=sr[:], in_=r[:], axis=AX.X)
                prf = pool_p.tile([T, T], BF16, tag="prf", name="prf",
                                  bufs=5)
                nc.vector.scalar_tensor_tensor(
                    out=prf[:], in0=fT[:], scalar=sr[:], in1=Ef[:],
                    op0=ALU.mult, op1=ALU.mult)
                # r-scale split DVE / Pool
                nc.vector.tensor_tensor(
                    out=M[:, 0:CTBK, :], in0=M[:, 0:CTBK, :],
                    in1=_bcast_last(r[:, 0:CTBK], 128), op=ALU.mult)
                nc.gpsimd.tensor_tensor(
                    out=M[:, CTBK:K, :], in0=M[:, CTBK:K, :],
                    in1=_bcast_last(r[:, CTBK:K], 128), op=ALU.mult)
                # k-sum: accumulate slices 1..15 onto slice 0
                if KN_KSUM == "dma":
                    nc.gpsimd.dma_start(
                        out=M[:, 0:1, :].to_broadcast([T, K - 1, 128]),
                        in_=M[:, 1:K, :], accum_op=ALU.add)
                else:
                    nc.gpsimd.tensor_add(out=M[:, 0:8, :], in0=M[:, 0:8, :],
                                         in1=M[:, 8:16, :])
                    nc.gpsimd.tensor_add(out=M[:, 0:4, :], in0=M[:, 0:4, :],
                                         in1=M[:, 4:8, :])
                    nc.gpsimd.tensor_add(out=M[:, 0:2, :], in0=M[:, 0:2, :],
                                         in1=M[:, 2:4, :])
                    nc.gpsimd.tensor_add(out=M[:, 0, :], in0=M[:, 0, :],
                                         in1=M[:, 1, :])
                return (M, prf)

            # ================= main loop =================
            for st in range(nst):
                g0 = st * S
                ft = feat_p.tile([128, 1, FW], BF16, tag="ft", name="ft")
                nc.sync.dma_start(
                    out=ft[:, 0, :], in_=featT[:, g0 * T:(g0 + S) * T])
                m1h = feat_p.tile([128, 1, FW], BF16, tag="m1h", name="m1h")
                fmlp([ft[:, 0, :]], "m1_W1", "m1_b1", 1, 128, "relu", m1h)
                x1 = feat_p.tile([128, 1, FW], BF16, tag="x1", name="x1")
                fmlp([m1h[:, 0, :]], "m1_W2", "m1_b2", 1, 128, "ident", x1)
                x1 = x1[:, 0, :]

                # --- pass B: L1(both) + enc1 + pool1 ---
                hts = []
                p1lo = feat_p.tile([128, FW], BF16, tag="p1lo", name="p1lo")
                p1hi = feat_p.tile([128, FW], BF16, tag="p1hi", name="p1hi")
                geos = []
                for t in range(S):
                    geo_t = geo_p.tile([11, K, T], BF16, tag="geo",
                                       name="geo_t", bufs=8)
                    nc.sync.dma_start(out=geo_t[:], in_=geo4[g0 + t])
                    geos.append(geo_t)
                prq = []
                for t in range(S):
                    h_t = h_p.tile([128, K, T], BF16, tag="h", name="h_t")
                    enc_l1(geos[t], h_t)
                    hts.append(h_t)
                    if len(prq) >= 2:
                        pool_out(prq[-2], t - 2, p1lo, p1hi)
                    prq.append(enc_pool("l1", 0, x1, t, h_t))
                for t in range(S - 2, S):
                    pool_out(prq[t], t, p1lo, p1hi)
                p1h = feat_p.tile([128, 2, FW], BF16, tag="p1h", name="p1h")
                fmlp([p1lo, p1hi], "p1_W1", "p1_b1", 2, 256, "relu", p1h)
                x2 = feat_p.tile([128, 1, FW], BF16, tag="x2", name="x2")
                fmlp([p1h[:, 0, :], p1h[:, 1, :]], "p1_W2", "p1_b2", 2, 128,
                     "ident", x2)
                x2 = x2[:, 0, :]

                # --- pass C: enc2 + pool2 (reuses h rows 64-127) ---
                p2lo = feat_p.tile([128, FW], BF16, tag="p2lo", name="p2lo")
                p2hi = feat_p.tile([128, FW], BF16, tag="p2hi", name="p2hi")
                prq = []
                for t in range(S):
                    if len(prq) >= 2:
                        pool_out(prq[-2], t - 2, p2lo, p2hi)
                    prq.append(enc_pool("l2", 1, x2, t, hts[t]))
                for t in range(S - 2, S):
                    pool_out(prq[t], t, p2lo, p2hi)
                p2h = feat_p.tile([128, 2, FW], BF16, tag="p2h", name="p2h")
                fmlp([p2lo, p2hi], "p2_W1", "p2_b1", 2, 256, "relu", p2h)
                x3 = feat_p.tile([128, 2, FW], BF16, tag="x3", name="x3")
                fmlp([p2h[:, 0, :], p2h[:, 1, :]], "p2_W2", "p2_b2", 2, 256,
                     "ident", x3)

                # --- m2(x3) + m3(features) + leaky ---
                m2h = feat_p.tile([128, 3, FW], BF16, tag="m2h", name="m2h")
                fmlp([x3[:, 0, :], x3[:, 1, :]], "m2_W1", "m2_b1", 2, 384,
                     "relu", m2h)
                m3h = feat_p.tile([128, 2, FW], BF16, tag="m3h", name="m3h")
                fmlp([ft[:, 0, :]], "m3_W1", "m3_b1", 1, 256, "relu", m3h)
                osb = feat_p.tile([128, 4, FW], BF16, tag="osb", name="osb")
                fmlp([m2h[:, 0, :], m2h[:, 1, :], m2h[:, 2, :]],
                     "m2_W2", "b2s", 3, 512, "prelu", osb,
                     extra=([m3h[:, 0, :], m3h[:, 1, :]], "m3_W2", 2))
                nc.sync.dma_start(out=out_d[:, :, st, :], in_=osb[:])

    nc.finalize()
    return nc


_BUILD_CACHE = {}


def _get_prog(nsh_pad):
    if nsh_pad not in _BUILD_CACHE:
        _BUILD_CACHE[nsh_pad] = build(nsh_pad)
    return _BUILD_CACHE[nsh_pad]


def _prep_weights(i):
    """Host-side weight packing -> dict of arrays (shared across cores)."""
    o = {}

    def blkpack(W, cout):
        cin = W.shape[0]
        nci = cin // 128
        return np.concatenate([W[ci * 128:(ci + 1) * 128, :]
                               for ci in range(nci)], axis=1)

    def bpack(b):
        nblk = b.shape[0] // 128
        return np.ascontiguousarray(b.reshape(nblk, 128).T)

    perm = [6, 7, 8, 9, 0, 1, 2, 3, 4, 5]   # [rel, dist, orig, nbr]
    w1s = []
    w2s = []
    for ln in ("l1", "l2"):
        W1, b1 = i[ln + "_W1"], i[ln + "_b1"]
        W2, b2 = i[ln + "_W2"], i[ln + "_b2"]
        w1a = np.concatenate([W1[perm, :], b1[None, :]], 0)  # [11, 64]
        w1s.append(w1a)
        w2s.append(W2)
        o[ln + "_b2q"] = np.tile(b2, 4)[None, :].astype(BF)
    o["w1a_cat"] = np.concatenate(w1s, axis=1).astype(BF)    # [11, 128]
    o["w2a_cat"] = np.concatenate(w2s, axis=0).astype(BF)    # [128, 128]
    o["m1_W1"] = i["m1_W1"].astype(BF)
    o["m1_W2"] = i["m1_W2"].astype(BF)
    o["m1_b1"] = i["m1_b1"].reshape(128, 1).astype(np.float32)
    o["m1_b2"] = i["m1_b2"].reshape(128, 1).astype(np.float32)
    o["p1_W1"] = blkpack(i["p1_W1"], 256).astype(BF)
    o["p1_b1"] = bpack(i["p1_b1"]).astype(np.float32)
    o["p1_W2"] = blkpack(i["p1_W2"], 128).astype(BF)
    o["p1_b2"] = i["p1_b2"].reshape(128, 1).astype(np.float32)
    o["p2_W1"] = blkpack(i["p2_W1"], 256).astype(BF)
    o["p2_b1"] = bpack(i["p2_b1"]).astype(np.float32)
    o["p2_W2"] = blkpack(i["p2_W2"], 256).astype(BF)
    o["p2_b2"] = bpack(i["p2_b2"]).astype(np.float32)
    o["m2_W1"] = blkpack(i["m2_W1"], 384).astype(BF)
    o["m2_b1"] = bpack(i["m2_b1"]).astype(np.float32)
    o["m2_W2"] = blkpack(i["m2_W2"], 512).astype(BF)
    o["m3_W1"] = blkpack(i["m3_W1"], 256).astype(BF)
    o["m3_b1"] = bpack(i["m3_b1"]).astype(np.float32)
    o["m3_W2"] = blkpack(i["m3_W2"], 512).astype(BF)
    o["b2s"] = bpack(i["m2_b2"] + i["m3_b2"]).astype(np.float32)
    return o


def _prep_core(coords, features, neighbor_idx, c0, c1, nsh_pad):
    nsh = c1 - c0
    pad = nsh_pad - nsh
    nt = nsh_pad // T
    feat = features[c0:c1]
    if pad:
        feat = np.concatenate(
            [feat, np.zeros((pad, feat.shape[1]), np.float32)], 0)
    featT = np.ascontiguousarray(feat.T).astype(BF)
    cs = coords[c0:c1]
    nbr = coords[neighbor_idx[c0:c1]]            # (nsh, K, 3)
    if pad:
        cs = np.concatenate([cs, np.zeros((pad, 3), np.float32)], 0)
        nbr = np.concatenate(
            [nbr, np.zeros((pad, K, 3), np.float32)], 0)
    orig = np.broadcast_to(cs[:, None, :], nbr.shape)      # (np, K, 3)
    rel = orig - nbr
    dist = np.sqrt((rel * rel).sum(-1))                    # (np, K)
    geo = np.empty((nsh_pad, K, 11), np.float32)
    geo[:, :, 0:3] = rel
    geo[:, :, 3] = dist
    geo[:, :, 4:7] = orig
    geo[:, :, 7:10] = nbr
    geo[:, :, 10] = 1.0
    # -> [nt, 11, K, T]
    geo4 = np.ascontiguousarray(
        geo.reshape(nt, T, K, 11).transpose(0, 3, 2, 1)).astype(BF)
    return {"featT": featT, "geo4": geo4}


def prepare_in_maps(inputs, nsh_pad):
    coords = np.asarray(inputs["coords"], np.float32)
    features = np.asarray(inputs["features"], np.float32)
    idx = np.asarray(inputs["neighbor_idx"])
    wmaps = _prep_weights({k: np.asarray(v, np.float32)
                           for k, v in inputs.items()
                           if k not in ("coords", "features", "neighbor_idx")})
    in_maps = []
    for c in range(NCORES):
        m = dict(wmaps)
        m.update(_prep_core(coords, features, idx,
                            c * NSH, (c + 1) * NSH, nsh_pad))
        in_maps.append(m)
    return in_maps


def assemble_out(results, nsh_pad):
    outs = []
    for c in range(NCORES):
        r = np.asarray(results[c]["out"]).astype(np.float32)  # [128,4,nst,FW]
        r = np.transpose(r, (2, 3, 1, 0)).reshape(nsh_pad, 512)[:NSH]
        outs.append(r)
    return np.ascontiguousarray(np.concatenate(outs, 0))


LAST_RES = None


def kernel(**inputs):
    global LAST_RES
    nsh_pad = _ceil_to(NSH, T * S)               # 12544
    nc = _get_prog(nsh_pad)
    in_maps = prepare_in_maps(inputs, nsh_pad)
    trace = bool(os.environ.get("KERNEL_TRACE"))
    res = run_bass_kernel_spmd(nc, in_maps, core_ids=list(range(NCORES)),
                               trace=trace)
    LAST_RES = res
    return assemble_out(res.results, nsh_pad)


if __name__ == "__main__":
    rng = np.random.default_rng(0)
    inp = {
        "coords": rng.standard_normal((N_FULL, 3)).astype(np.float32),
        "features": rng.standard_normal((N_FULL, 128)).astype(np.float32),
        "neighbor_idx": rng.integers(0, N_FULL, (N_FULL, 16), dtype=np.int32),
    }
    for nm, ci, ch, co in [("m1", 128, 128, 128), ("m2", 256, 384, 512),
                           ("m3", 128, 256, 512), ("l1", 10, 64, 128),
                           ("l2", 10, 64, 128), ("p1", 256, 256, 128),
                           ("p2", 256, 256, 256)]:
        inp[nm + "_W1"] = rng.standard_normal((ci, ch)).astype(np.float32)
        inp[nm + "_b1"] = rng.standard_normal(ch).astype(np.float32)
        inp[nm + "_W2"] = rng.standard_normal((ch, co)).astype(np.float32)
        inp[nm + "_b2"] = rng.standard_normal(co).astype(np.float32)
    out = kernel(**inp)
    print("out", out.shape, out.dtype)


# revision 25
# speedup vs baseline: 1.0740x; 1.0740x over previous
"""Trainium2 Bass kernel for LocalFeatureAggregation (gnn_message_passing).

Sharding: data-parallel over points. Each of the 8 cores gets 12500 points
(padded to 12544 = 98 tiles of 128). All neighbor geometry (rel, dist, orig,
nbr, ones-bias-row) is precomputed on host and shipped as an 11-row tensor
per tile; MLP weights replicated.

Math decomposition (same as validated v1):
  lse+pool(x, f): z = [enc | f], softmax over channels, sum over K.
    E=exp(z); s_k = sum_c E; Ef=exp(f); Sf=sum_c Ef; r_k = 1/(s_k+Sf)
    pool_enc = sum_k z*E*r_k ; pool_f = (f*Ef) * sum_k r_k

v2 structure:
  - geo [11,K,T] from host (incl dist + ones row) -> L1 of BOTH encoders
    fused into one [11->128] matmul pair + one relu per tile.
  - z as a single [128,16,128] f32 PSUM tile (16 matmuls + 4 bias matmuls).
  - one exp over the full z; Sf via activation accum_out.
  - per-k tensor_scalar r-scaling (4x DVE mode); k-sum tree split
    DVE/Pool; f-half via fused scalar_tensor_tensor.
  - final leaky via Prelu activation (bias=b2s, alpha=0.01).
  - all activation funcs in one table set (exp_and_others) -> 1 table load.
"""
import os
import numpy as np
import ml_dtypes

import concourse.bass as bass
import concourse.bacc as bacc
import concourse.mybir as mybir
import concourse.tile as tile
from concourse.bass_utils import run_bass_kernel_spmd
from concourse.masks import make_identity

AF = mybir.ActivationFunctionType
ALU = mybir.AluOpType
AX = mybir.AxisListType
F32 = mybir.dt.float32
BF16 = mybir.dt.bfloat16
BF = ml_dtypes.bfloat16

N_FULL = 100000
NCORES = 8
NSH = N_FULL // NCORES      # 12500
T = 128                     # points per tile
K = 16                      # neighbors
S = 7                       # tiles per supertile
D_IN = 128
D_OUT = 256

# engine knobs for flexible ops
KN_IDENT = "act"            # feature-path identity moves: 'act'|'dve'
KN_KSUM = "dma"             # k-sum: 'dma' (gpsimd accum dma) | 'pool' tree
CTBK = 6                    # ctb k-slices on DVE (rest on Pool)


def _ceil_to(x, m):
    return ((x + m - 1) // m) * m


def _bcast_last(ap2d, last):
    """[P, F] AP -> [P, F, last] AP with step-0 trailing dim."""
    a = ap2d.rearrange("p (f a) -> p f a", a=1)
    return a.to_broadcast([ap2d.shape[0], ap2d.shape[1], last])


def build(nsh_pad):
    assert nsh_pad % (T * S) == 0
    nt = nsh_pad // T           # tiles
    nst = nt // S               # supertiles
    FW = S * T                  # 896
    CH = [(0, 448), (448, 448)]

    nc = bacc.Bacc(trn_type="TRN2")

    featT = nc.dram_tensor("featT", [D_IN, nsh_pad], BF16, kind="ExternalInput")
    geo4 = nc.dram_tensor("geo4", [nt, 11, K, T], BF16, kind="ExternalInput")
    out_d = nc.dram_tensor("out", [128, 4, nst, FW], BF16,
                           kind="ExternalOutput")

    def din(name, shape, dt):
        return nc.dram_tensor(name, shape, dt, kind="ExternalInput")

    wd = {}
    wd["w1a_cat"] = din("w1a_cat", [11, 128], BF16)
    wd["w2a_cat"] = din("w2a_cat", [128, 128], BF16)
    for ln in ("l1", "l2"):
        wd[ln + "_b2q"] = din(ln + "_b2q", [1, 512], BF16)
    wd["m1_W1"] = din("m1_W1", [128, 128], BF16)
    wd["m1_W2"] = din("m1_W2", [128, 128], BF16)
    wd["m1_b1"] = din("m1_b1", [128, 1], F32)
    wd["m1_b2"] = din("m1_b2", [128, 1], F32)
    wd["p1_W1"] = din("p1_W1", [128, 512], BF16)
    wd["p1_b1"] = din("p1_b1", [128, 2], F32)
    wd["p1_W2"] = din("p1_W2", [128, 256], BF16)
    wd["p1_b2"] = din("p1_b2", [128, 1], F32)
    wd["p2_W1"] = din("p2_W1", [128, 512], BF16)
    wd["p2_b1"] = din("p2_b1", [128, 2], F32)
    wd["p2_W2"] = din("p2_W2", [128, 512], BF16)
    wd["p2_b2"] = din("p2_b2", [128, 2], F32)
    wd["m2_W1"] = din("m2_W1", [128, 768], BF16)
    wd["m2_b1"] = din("m2_b1", [128, 3], F32)
    wd["m2_W2"] = din("m2_W2", [128, 1536], BF16)
    wd["m3_W1"] = din("m3_W1", [128, 256], BF16)
    wd["m3_b1"] = din("m3_b1", [128, 2], F32)
    wd["m3_W2"] = din("m3_W2", [128, 1024], BF16)
    wd["b2s"] = din("b2s", [128, 4], F32)

    with tile.TileContext(nc) as tc:
        import contextlib
        with contextlib.ExitStack() as ctx:
            cst = ctx.enter_context(tc.tile_pool(name="cst", bufs=1))
            geo_p = ctx.enter_context(tc.tile_pool(name="geo", bufs=4))
            h_p = ctx.enter_context(tc.tile_pool(name="hp", bufs=8))
            enc_p = ctx.enter_context(tc.tile_pool(name="enc", bufs=2))
            pool_p = ctx.enter_context(tc.tile_pool(name="pool", bufs=3))
            feat_p = ctx.enter_context(tc.tile_pool(name="feat", bufs=2))
            ps_z = ctx.enter_context(
                tc.tile_pool(name="psz", bufs=1, space="PSUM"))
            ps_h = ctx.enter_context(
                tc.tile_pool(name="psh", bufs=1, space="PSUM"))
            ps_m = ctx.enter_context(
                tc.tile_pool(name="psm", bufs=2, space="PSUM"))

            # ---------- constants ----------
            w_sb = {}
            for name, dr in wd.items():
                t_ = cst.tile(list(dr.shape), dr.dtype, name="w_" + name)
                nc.sync.dma_start(out=t_[:], in_=dr[:])
                w_sb[name] = t_
            ones_col = cst.tile([1, 128], BF16, name="ones_col")
            nc.gpsimd.memset(ones_col[:], 1.0)
            ident = cst.tile([128, 128], BF16, name="ident")
            make_identity(nc, ident[:])

            def fmlp(rhs_tiles, wkey, bkey, nci, ncout, kind, out_t,
                     extra=None):
                """Channel-major MLP layer over the supertile.
                rhs_tiles: list of nci tiles [128, FW]; out_t [128, nblk, FW].
                kind: 'relu' | 'ident' | 'prelu'.
                extra: (rhs_tiles2, wkey2, nci2) accumulated into same psum."""
                w = w_sb[wkey]
                b = w_sb[bkey]
                nblk = ncout // 128
                for co in range(nblk):
                    for c0, cw in CH:
                        ps = ps_m.tile([128, 448], F32, tag="misc", name="ps_f")
                        tot = nci + (extra[2] if extra else 0)
                        i = 0
                        for ci in range(nci):
                            nc.tensor.matmul(
                                out=ps[:, :cw],
                                lhsT=w[:, ci * ncout + co * 128:
                                       ci * ncout + (co + 1) * 128],
                                rhs=rhs_tiles[ci][:, c0:c0 + cw],
                                start=(i == 0), stop=(i == tot - 1))
                            i += 1
                        if extra:
                            rhs2, wkey2, nci2 = extra
                            w2 = w_sb[wkey2]
                            for ci in range(nci2):
                                nc.tensor.matmul(
                                    out=ps[:, :cw],
                                    lhsT=w2[:, ci * ncout + co * 128:
                                            ci * ncout + (co + 1) * 128],
                                    rhs=rhs2[ci][:, c0:c0 + cw],
                                    start=(i == 0), stop=(i == tot - 1))
                                i += 1
                        dst = out_t[:, co, c0:c0 + cw]
                        if kind == "relu":
                            nc.scalar.activation(
                                out=dst, in_=ps[:, :cw], func=AF.Relu,
                                bias=b[:, co:co + 1])
                        elif kind == "prelu":
                            nc.scalar.activation(
                                out=dst, in_=ps[:, :cw], func=AF.Prelu,
                                bias=b[:, co:co + 1], alpha=0.01)
                        elif KN_IDENT == "act":
                            nc.scalar.activation(
                                out=dst, in_=ps[:, :cw], func=AF.Identity,
                                bias=b[:, co:co + 1])
                        else:
                            nc.vector.tensor_scalar_add(
                                out=dst, in0=ps[:, :cw],
                                scalar1=b[:, co:co + 1])

            def enc_l1(geo_t, h_t):
                """Fused L1 for both encoders: h_t [128, K, T] bf16.
                Rows 0-63 = l1 hidden, 64-127 = l2 hidden."""
                for half in range(2):
                    ph = ps_h.tile([128, 8, T], F32, tag="ph", name="ph")
                    for q in range(2):
                        nc.tensor.matmul(
                            out=ph[:, 4 * q:4 * q + 4, :],
                            lhsT=w_sb["w1a_cat"][:],
                            rhs=geo_t[:, 8 * half + 4 * q:
                                      8 * half + 4 * q + 4, :])
                    nc.scalar.activation(
                        out=h_t[:, 8 * half:8 * half + 8, :], in_=ph[:],
                        func=AF.Relu)

            def pool_out(res, t_in_s, pcm_lo, pcm_hi):
                """XBAR-transpose pool rows -> channel-major dest slices."""
                M, prf = res
                dst = pcm_lo[:, t_in_s * T:(t_in_s + 1) * T]
                nc.sync.dma_start(out=dst, in_=M[:, 0, :], transpose=True)
                dst = pcm_hi[:, t_in_s * T:(t_in_s + 1) * T]
                nc.sync.dma_start(out=dst, in_=prf[:], transpose=True)

            def enc_pool(lname, half, xcm, t_in_s, h_t):
                """Encoder L2 + attentive pool for one tile.
                Returns (M, prf): M[:,0,:] = enc-half pool row (point-major),
                prf = f-half pool row."""
                w2a = w_sb["w2a_cat"][64 * half:64 * half + 64, :]
                b2q = w_sb[lname + "_b2q"]
                hh = h_t[64 * half:64 * half + 64, :, :]
                # f-half first: fT early so ACT's Ef can overlap L2
                fT = ps_m.tile([T, 128], BF16, tag="misc", name="fT")
                nc.tensor.transpose(
                    out=fT[:], in_=xcm[:, t_in_s * T:(t_in_s + 1) * T],
                    identity=ident[:])
                Ef = pool_p.tile([T, 128], BF16, tag="Ef", name="Ef")
                Sf = pool_p.tile([T, 1], F32, tag="Sf", name="Sf")
                with nc.allow_low_precision("accum f32 ok"):
                    nc.scalar.activation(out=Ef[:], in_=fT[:], func=AF.Exp,
                                         accum_out=Sf[:])
                E = enc_p.tile([T, K, 128], BF16, tag="E", name="E")
                M = enc_p.tile([T, K, 128], BF16, tag="M", name="M", bufs=4)
                for hf in range(2):
                    z = ps_z.tile([T, 8, 128], F32, tag=f"z{hf}", name="z")
                    for k in range(8):
                        nc.tensor.matmul(
                            out=z[:, k, :], lhsT=hh[:, 8 * hf + k, :],
                            rhs=w2a, start=True, stop=False,
                            skip_group_check=True)
                    for g in range(2):
                        nc.tensor.matmul(
                            out=z[:, 4 * g:4 * g + 4, :], lhsT=ones_col[:],
                            rhs=b2q[:], start=False, stop=True,
                            skip_group_check=True)
                    sl = slice(8 * hf, 8 * hf + 8)
                    nc.scalar.activation(out=E[:, sl, :], in_=z[:],
                                         func=AF.Exp)
                    nc.vector.tensor_mul(out=M[:, sl, :], in0=E[:, sl, :],
                                         in1=z[:])
                # s_k = sum_c E via two bf16 halvings + short reduce
                s = pool_p.tile([T, K], BF16, tag="s", name="s")
                sc = pool_p.tile([T, K, 64], BF16, tag="sc", name="sc")
                with nc.allow_low_precision("bf16 softmax denom"):
                    nc.vector.tensor_add(out=sc[:], in0=E[:, :, 0:64],
                                         in1=E[:, :, 64:128])
                    nc.vector.tensor_add(out=sc[:, :, 0:32],
                                         in0=sc[:, :, 0:32],
                                         in1=sc[:, :, 32:64])
                    nc.vector.reduce_sum(out=s[:], in_=sc[:, :, 0:32],
                                         axis=AX.X)
                u = pool_p.tile([T, K], BF16, tag="u", name="u")
                with nc.allow_low_precision("bf16 softmax denom"):
                    nc.scalar.activation(out=u[:], in_=s[:],
                                         func=AF.Identity, bias=Sf[:])
                r = pool_p.tile([T, K], F32, tag="r", name="r")
                nc.vector.reciprocal(out=r[:], in_=u[:])
                sr = pool_p.tile([T, 1], F32, tag="sr", name="sr")
                nc.vector.reduce_sum(out=sr[:], in_=r[:], axis=AX.X)
                prf = pool_p.tile([T, T], BF16, tag="prf", name="prf",
                                  bufs=5)
                nc.vector.scalar_tensor_tensor(
                    out=prf[:], in0=fT[:], scalar=sr[:], in1=Ef[:],
                    op0=ALU.mult, op1=ALU.mult)
                # r-scale split DVE / Pool
                nc.vector.tensor_tensor(
                    out=M[:, 0:CTBK, :], in0=M[:, 0:CTBK, :],
                    in1=_bcast_last(r[:, 0:CTBK], 128), op=ALU.mult)
                nc.gpsimd.tensor_tensor(
                    out=M[:, CTBK:K, :], in0=M[:, CTBK:K, :],
                    in1=_bcast_last(r[:, CTBK:K], 128), op=ALU.mult)
                # k-sum: accumulate slices 1..15 onto slice 0
                if KN_KSUM == "dma":
                    nc.gpsimd.dma_start(
                        out=M[:, 0:1, :].to_broadcast([T, K - 1, 128]),
                        in_=M[:, 1:K, :], accum_op=ALU.add)
                else:
                    nc.gpsimd.tensor_add(out=M[:, 0:8, :], in0=M[:, 0:8, :],
                                         in1=M[:, 8:16, :])
                    nc.gpsimd.tensor_add(out=M[:, 0:4, :], in0=M[:, 0:4, :],
                                         in1=M[:, 4:8, :])
                    nc.gpsimd.tensor_add(out=M[:, 0:2, :], in0=M[:, 0:2, :],
                                         in1=M[:, 2:4, :])
                    nc.gpsimd.tensor_add(out=M[:, 0, :], in0=M[:, 0, :],
                                         in1=M[:, 1, :])
                return (M, prf)

            # ================= main loop =================
            for st in range(nst):
                g0 = st * S
                ft = feat_p.tile([128, 1, FW], BF16, tag="ft", name="ft")
                nc.sync.dma_start(
                    out=ft[:, 0, :], in_=featT[:, g0 * T:(g0 + S) * T])
                m1h = feat_p.tile([128, 1, FW], BF16, tag="m1h", name="m1h")
                fmlp([ft[:, 0, :]], "m1_W1", "m1_b1", 1, 128, "relu", m1h)
                x1 = feat_p.tile([128, 1, FW], BF16, tag="x1", name="x1")
                fmlp([m1h[:, 0, :]], "m1_W2", "m1_b2", 1, 128, "ident", x1)
                x1 = x1[:, 0, :]

                # --- pass B: L1(both) + enc1 + pool1 ---
                hts = []
                p1lo = feat_p.tile([128, FW], BF16, tag="p1lo", name="p1lo")
                p1hi = feat_p.tile([128, FW], BF16, tag="p1hi", name="p1hi")
                geos = []
                for t in range(S):
                    geo_t = geo_p.tile([11, K, T], BF16, tag="geo",
                                       name="geo_t", bufs=8)
                    nc.sync.dma_start(out=geo_t[:], in_=geo4[g0 + t])
                    geos.append(geo_t)
                prq = []
                for t in range(S):
                    h_t = h_p.tile([128, K, T], BF16, tag="h", name="h_t")
                    enc_l1(geos[t], h_t)
                    hts.append(h_t)
                    if len(prq) >= 2:
                        pool_out(prq[-2], t - 2, p1lo, p1hi)
                    prq.append(enc_pool("l1", 0, x1, t, h_t))
                for t in range(S - 2, S):
                    pool_out(prq[t], t, p1lo, p1hi)
                p1h = feat_p.tile([128, 2, FW], BF16, tag="p1h", name="p1h")
                fmlp([p1lo, p1hi], "p1_W1", "p1_b1", 2, 256, "relu", p1h)
                x2 = feat_p.tile([128, 1, FW], BF16, tag="x2", name="x2")
                fmlp([p1h[:, 0, :], p1h[:, 1, :]], "p1_W2", "p1_b2", 2, 128,
                     "ident", x2)
                x2 = x2[:, 0, :]

                # --- pass C: enc2 + pool2 (reuses h rows 64-127) ---
                p2lo = feat_p.tile([128, FW], BF16, tag="p2lo", name="p2lo")
                p2hi = feat_p.tile([128, FW], BF16, tag="p2hi", name="p2hi")
                prq = []
                for t in range(S):
                    if len(prq) >= 2:
                        pool_out(prq[-2], t - 2, p2lo, p2hi)
                    prq.append(enc_pool("l2", 1, x2, t, hts[t]))
                for t in range(S - 2, S):
                    pool_out(prq[t], t, p2lo, p2hi)
                p2h = feat_p.tile([128, 2, FW], BF16, tag="p2h", name="p2h")
                fmlp([p2lo, p2hi], "p2_W1", "p2_b1", 2, 256, "relu", p2h)
                x3 = feat_p.tile([128, 2, FW], BF16, tag="x3", name="x3")
                fmlp([p2h[:, 0, :], p2h[:, 1, :]], "p2_W2", "p2_b2", 2, 256,
                     "ident", x3)

                # --- m2(x3) + m3(features) + leaky ---
                m2h = feat_p.tile([128, 3, FW], BF16, tag="m2h", name="m2h")
                fmlp([x3[:, 0, :], x3[:, 1, :]], "m2_W1", "m2_b1", 2, 384,
                     "relu", m2h)
                m3h = feat_p.tile([128, 2, FW], BF16, tag="m3h", name="m3h")
                fmlp([ft[:, 0, :]], "m3_W1", "m3_b1", 1, 256, "relu", m3h)
                osb = feat_p.tile([128, 4, FW], BF16, tag="osb", name="osb")
                fmlp([m2h[:, 0, :], m2h[:, 1, :], m2h[:, 2, :]],
                     "m2_W2", "b2s", 3, 512, "prelu", osb,
                     extra=([m3h[:, 0, :], m3h[:, 1, :]], "m3_W2", 2))
                nc.sync.dma_start(out=out_d[:, :, st, :], in_=osb[:])

    nc.finalize()
    return nc


_BUILD_CACHE = {}


def _get_prog(nsh_pad):
    if nsh_pad not in _BUILD_CACHE:
        _BUILD_CACHE[nsh_pad] = build(nsh_pad)
    return _BUILD_CACHE[nsh_pad]


def _prep_weights(i):
    """Host-side weight packing -> dict of arrays (shared across cores)."""
    o = {}

    def blkpack(W, cout):
        cin = W.shape[0]
        nci = cin // 128
        return np.concatenate([W[ci * 128:(ci + 1) * 128, :]
                               for ci in range(nci)], axis=1)

    def bpack(b):
        nblk = b.shape[0] // 128
        return np.ascontiguousarray(b.reshape(nblk, 128).T)

    perm = [6, 7, 8, 9, 0, 1, 2, 3, 4, 5]   # [rel, dist, orig, nbr]
    w1s = []
    w2s = []
    for ln in ("l1", "l2"):
        W1, b1 = i[ln + "_W1"], i[ln + "_b1"]
        W2, b2 = i[ln + "_W2"], i[ln + "_b2"]
        w1a = np.concatenate([W1[perm, :], b1[None, :]], 0)  # [11, 64]
        w1s.append(w1a)
        w2s.append(W2)
        o[ln + "_b2q"] = np.tile(b2, 4)[None, :].astype(BF)
    o["w1a_cat"] = np.concatenate(w1s, axis=1).astype(BF)    # [11, 128]
    o["w2a_cat"] = np.concatenate(w2s, axis=0).astype(BF)    # [128, 128]
    o["m1_W1"] = i["m1_W1"].astype(BF)
    o["m1_W2"] = i["m1_W2"].astype(BF)
    o["m1_b1"] = i["m1_b1"].reshape(128, 1).astype(np.float32)
    o["m1_b2"] = i["m1_b2"].reshape(128, 1).astype(np.float32)
    o["p1_W1"] = blkpack(i["p1_W1"], 256).astype(BF)
    o["p1_b1"] = bpack(i["p1_b1"]).astype(np.float32)
    o["p1_W2"] = blkpack(i["p1_W2"], 128).astype(BF)
    o["p1_b2"] = i["p1_b2"].reshape(128, 1).astype(np.float32)
    o["p2_W1"] = blkpack(i["p2_W1"], 256).astype(BF)
    o["p2_b1"] = bpack(i["p2_b1"]).astype(np.float32)
    o["p2_W2"] = blkpack(i["p2_W2"], 256).astype(BF)
    o["p2_b2"] = bpack(i["p2_b2"]).astype(np.float32)
    o["m2_W1"] = blkpack(i["m2_W1"], 384).astype(BF)
    o["m2_b1"] = bpack(i["m2_b1"]).astype(np.float32)
    o["m2_W2"] = blkpack(i["m2_W2"], 512).astype(BF)
    o["m3_W1"] = blkpack(i["m3_W1"], 256).astype(BF)
    o["m3_b1"] = bpack(i["m3_b1"]).astype(np.float32)
    o["m3_W2"] = blkpack(i["m3_W2"], 512).astype(BF)
    o["b2s"] = bpack(i["m2_b2"] + i["m3_b2"]).astype(np.float32)
    return o


def _prep_core(coords, features, neighbor_idx, c0, c1, nsh_pad):
    nsh = c1 - c0
    pad = nsh_pad - nsh
    nt = nsh_pad // T
    feat = features[c0:c1]
    if pad:
        feat = np.concatenate(
            [feat, np.zeros((pad, feat.shape[1]), np.float32)], 0)
    featT = np.ascontiguousarray(feat.T).astype(BF)
    cs = coords[c0:c1]
    nbr = coords[neighbor_idx[c0:c1]]            # (nsh, K, 3)
    if pad:
        cs = np.concatenate([cs, np.zeros((pad, 3), np.float32)], 0)
        nbr = np.concatenate(
            [nbr, np.zeros((pad, K, 3), np.float32)], 0)
    orig = np.broadcast_to(cs[:, None, :], nbr.shape)      # (np, K, 3)
    rel = orig - nbr
    dist = np.sqrt((rel * rel).sum(-1))                    # (np, K)
    geo = np.empty((nsh_pad, K, 11), np.float32)
    geo[:, :, 0:3] = rel
    geo[:, :, 3] = dist
    geo[:, :, 4:7] = orig
    geo[:, :, 7:10] = nbr
    geo[:, :, 10] = 1.0
    # -> [nt, 11, K, T]
    geo4 = np.ascontiguousarray(
        geo.reshape(nt, T, K, 11).transpose(0, 3, 2, 1)).astype(BF)
    return {"featT": featT, "geo4": geo4}


def prepare_in_maps(inputs, nsh_pad):
    coords = np.asarray(inputs["coords"], np.float32)
    features = np.asarray(inputs["features"], np.float32)
    idx = np.asarray(inputs["neighbor_idx"])
    wmaps = _prep_weights({k: np.asarray(v, np.float32)
                           for k, v in inputs.items()
                           if k not in ("coords", "features", "neighbor_idx")})
    in_maps = []
    for c in range(NCORES):
        m = dict(wmaps)
        m.update(_prep_core(coords, features, idx,
                            c * NSH, (c + 1) * NSH, nsh_pad))
        in_maps.append(m)
    return in_maps


def assemble_out(results, nsh_pad):
    outs = []
    for c in range(NCORES):
        r = np.asarray(results[c]["out"]).astype(np.float32)  # [128,4,nst,FW]
        r = np.transpose(r, (2, 3, 1, 0)).reshape(nsh_pad, 512)[:NSH]
        outs.append(r)
    return np.ascontiguousarray(np.concatenate(outs, 0))


LAST_RES = None


def kernel(**inputs):
    global LAST_RES
    nsh_pad = _ceil_to(NSH, T * S)               # 12544
    nc = _get_prog(nsh_pad)
    in_maps = prepare_in_maps(inputs, nsh_pad)
    trace = bool(os.environ.get("KERNEL_TRACE"))
    res = run_bass_kernel_spmd(nc, in_maps, core_ids=list(range(NCORES)),
                               trace=trace)
    LAST_RES = res
    return assemble_out(res.results, nsh_pad)


if __name__ == "__main__":
    rng = np.random.default_rng(0)
    inp = {
        "coords": rng.standard_normal((N_FULL, 3)).astype(np.float32),
        "features": rng.standard_normal((N_FULL, 128)).astype(np.float32),
        "neighbor_idx": rng.integers(0, N_FULL, (N_FULL, 16), dtype=np.int32),
    }
    for nm, ci, ch, co in [("m1", 128, 128, 128), ("m2", 256, 384, 512),
                           ("m3", 128, 256, 512), ("l1", 10, 64, 128),
                           ("l2", 10, 64, 128), ("p1", 256, 256, 128),
                           ("p2", 256, 256, 256)]:
        inp[nm + "_W1"] = rng.standard_normal((ci, ch)).astype(np.float32)
        inp[nm + "_b1"] = rng.standard_normal(ch).astype(np.float32)
        inp[nm + "_W2"] = rng.standard_normal((ch, co)).astype(np.float32)
        inp[nm + "_b2"] = rng.standard_normal(co).astype(np.float32)
    out = kernel(**inp)
    print("out", out.shape, out.dtype)
